# revision 5
# baseline (speedup 1.0000x reference)
"""Bass/Trainium2 kernel for nn_DSF_Bern_I (gnn_message_passing).

Strategy
--------
N=4096 nodes is small, so the sparse scatter-add propagations are densified
into [4096,4096] adjacency matrices (built on host from edge_index) and all
message passing becomes TensorEngine matmuls across 8 NeuronCores:

* Nodes are row-sharded 512/core.  All big matmuls run in fp16 operands with
  fp32 PSUM accumulation (TensorE runs fp16 at full rate; fp16 matches the
  PE's internal fp32 rounding anyway).
* The K=10 Bernstein filter sum_j coeff_j*gamma_j*(I-A)^j(I+A)^{K-j} x0 is
  evaluated in the Krylov monomial basis v_k = A^k x0 (10 matrix applications
  instead of 65) and recombined with per-node gamma weights at the end.
* Each A-application computes the transposed shard v'[rows].T = v.T @ W.T
  with free dim 512, then PE-transposes back and AllGathers the fp16 shard.
* The pe-correlation chain keeps pe in transposed layout [64,512] per core;
  the dense sigmoid(c@c.T)@pe is chunked over 32 node chunks, pipelining
  PE matmul -> ACT sigmoid -> PE accumulate.  pe state / gamma gates are fp32.
* Per round, three fp16 AllGathers (v shard, c^T block, pe shard).

kernel(**inputs) takes the FULL inputs and returns (out, pe) like reference.
"""

import math
import os
import sys
from contextlib import ExitStack

sys.path.insert(0, "/opt/trn_rl_repo")

import numpy as np

R = 8  # cores
N = 4096
NB = N // R  # 512 rows per core
IN_DIM, HID_DIM, OUT_DIM = 1024, 512, 128
PE_IN, PE_HID, K = 128, 64, 10
PE_ALPHA, PE_BETA = 0.1, 0.5
NCHUNK = N // 128  # 32 node chunks
NBC = NB // 128  # 4 local row chunks

_last_exec_ns = None
_cache = {}


def _install_ntff_hook():
    import contextlib
    import ctypes
    import types

    if "antenv.axon_hooks" in sys.modules:
        return
    try:
        lib = ctypes.CDLL("/opt/axon/libaxon_pjrt.so")
    except OSError:
        return
    if not hasattr(lib, "axon_start_nrt_profile"):
        return
    lib.axon_start_nrt_profile.argtypes = [
        ctypes.POINTER(ctypes.c_int64),
        ctypes.c_size_t,
    ]
    lib.axon_start_nrt_profile.restype = ctypes.c_int64
    lib.axon_stop_nrt_profile.argtypes = [ctypes.c_char_p]
    lib.axon_stop_nrt_profile.restype = ctypes.c_int64

    @contextlib.contextmanager
    def _hook(output_dir, device_ids):
        import jax

        jax.devices()
        if device_ids:
            ids = (ctypes.c_int64 * len(device_ids))(*device_ids)
            rc = lib.axon_start_nrt_profile(ids, len(device_ids))
        else:
            rc = lib.axon_start_nrt_profile(None, 0)
        if rc != 0:
            raise RuntimeError(f"axon_start_nrt_profile rc={rc}")
        try:
            yield
        finally:
            n = lib.axon_stop_nrt_profile(str(output_dir).encode())
            print(f"profile: {n} file(s) in {output_dir}", file=sys.stderr)

    mod = types.ModuleType("antenv.axon_hooks")
    mod.get_axon_ntff_profile_hook = lambda: _hook
    mod.set_axon_ntff_profile_hook = lambda h: None
    sys.modules["antenv.axon_hooks"] = mod


def _build():
    import concourse.mybir as mybir
    import concourse.tile as tile
    from concourse import bacc

    F16 = mybir.dt.float16
    F32 = mybir.dt.float32
    AF = mybir.ActivationFunctionType
    Alu = mybir.AluOpType

    nc = bacc.Bacc("TRN2", target_bir_lowering=False, debug=False, num_devices=R)
    RG = [list(range(R))]

    def inp(name, shape, dt):
        return nc.dram_tensor(name, shape, dt, kind="ExternalInput")

    wlt = inp("wlt", [N, NB], F16)  # Wl[rows_b,:].T
    wgt = inp("wgt", [N, NB], F16)  # Wg[rows_b,:].T
    nft = inp("nft", [IN_DIM, NB], F16)  # node_feat[rows_b].T
    postf = inp("postf", [PE_IN, NB], F32)  # pos_enc[rows_b].T (fp32)
    w1 = inp("w1", [IN_DIM, HID_DIM], F16)
    w2 = inp("w2", [HID_DIM, OUT_DIM], F16)
    pew = inp("pew", [PE_IN, PE_HID], F32)
    corw = inp("corw", [PE_HID, PE_HID], F16)
    gatet = inp("gatet", [PE_HID, K + 1], F32)  # gate_w.T (fp32 for gamma)
    b1c = inp("b1c", [128, HID_DIM // 128], F32)  # lin1_b chunked per-partition
    b2c = inp("b2c", [128, 1], F32)
    pebc = inp("pebc", [PE_HID, 1], F32)
    corbc = inp("corbc", [PE_HID, 1], F32)
    gatebbc = inp("gatebbc", [128, K + 1], F32)  # gate_b broadcast over parts
    mrows = inp("mrows", [128, (K + 1) * (K + 1)], F32)  # Mt[j,:] bcast rows
    id128h = inp("id128h", [128, 128], F16)
    id64h = inp("id64h", [64, 64], F16)
    id64f = inp("id64f", [64, 64], F32)

    out_x = nc.dram_tensor("out_x", [NB, OUT_DIM], F32, kind="ExternalOutput")
    out_pe = nc.dram_tensor("out_pe", [NB, PE_HID], F32, kind="ExternalOutput")

    with tile.TileContext(nc) as tc, ExitStack() as pools:
        dram = pools.enter_context(tc.tile_pool(name="dram", bufs=1, space="DRAM"))
        stat = pools.enter_context(tc.tile_pool(name="stat", bufs=1))
        work = pools.enter_context(tc.tile_pool(name="work", bufs=2))
        ps = pools.enter_context(tc.tile_pool(name="ps", bufs=1, space="PSUM"))

        dma = nc.sync.dma_start
        mm = nc.tensor.matmul
        act = nc.scalar.activation
        vec = nc.vector

        # ---------- static SBUF loads ----------
        wlt_sb = stat.tile([128, NCHUNK * NB], F16, name="wlt_sb")
        dma(
            out=wlt_sb[:].rearrange("p (m d) -> p m d", m=NCHUNK),
            in_=wlt.ap().rearrange("(m p) d -> p m d", p=128),
        )
        wgt_sb = stat.tile([128, NCHUNK * NB], F16, name="wgt_sb")
        dma(
            out=wgt_sb[:].rearrange("p (m d) -> p m d", m=NCHUNK),
            in_=wgt.ap().rearrange("(m p) d -> p m d", p=128),
        )
        corw_sb = stat.tile([PE_HID, PE_HID], F16, name="corw_sb")
        dma(out=corw_sb[:], in_=corw.ap())
        gatet_sb = stat.tile([PE_HID, K + 1], F32, name="gatet_sb")
        dma(out=gatet_sb[:], in_=gatet.ap())
        b1_sb = stat.tile([128, HID_DIM // 128], F32, name="b1_sb")
        dma(out=b1_sb[:], in_=b1c.ap())
        b2_sb = stat.tile([128, 1], F32, name="b2_sb")
        dma(out=b2_sb[:], in_=b2c.ap())
        peb_sb = stat.tile([PE_HID, 1], F32, name="peb_sb")
        dma(out=peb_sb[:], in_=pebc.ap())
        corb_sb = stat.tile([PE_HID, 1], F32, name="corb_sb")
        dma(out=corb_sb[:], in_=corbc.ap())
        gateb_sb = stat.tile([128, K + 1], F32, name="gateb_sb")
        dma(out=gateb_sb[:], in_=gatebbc.ap())
        mrows_sb = stat.tile([128, (K + 1) * (K + 1)], F32, name="mrows_sb")
        dma(out=mrows_sb[:], in_=mrows.ap())
        id128h_sb = stat.tile([128, 128], F16, name="id128h_sb")
        dma(out=id128h_sb[:], in_=id128h.ap())
        id64h_sb = stat.tile([64, 64], F16, name="id64h_sb")
        dma(out=id64h_sb[:], in_=id64h.ap())
        id64f_sb = stat.tile([64, 64], F32, name="id64f_sb")
        dma(out=id64f_sb[:], in_=id64f.ap())

        # persistent krylov shards v_k[rows_b] fp16, chunk c at cols 128c
        v_keep = [
            stat.tile([128, NBC * 128], F16, name=f"vkeep{k}") for k in range(K + 1)
        ]
        # gamma gates, local rows chunk c: Gam[c][:, j]
        gam = [stat.tile([128, K + 1], F32, name=f"gam{c}") for c in range(NBC)]
        raw_sb = stat.tile([PE_HID, NB], F32, name="raw_sb")  # raw_pe.T fp32

        # ---------- encoder: x0T = (relu(nf@W1+b1)@W2+b2).T ----------
        w1_sb = work.tile([128, 8 * HID_DIM], F16, tag="w1sb", bufs=1)
        dma(
            out=w1_sb[:].rearrange("p (m d) -> p m d", m=8),
            in_=w1.ap().rearrange("(m p) d -> p m d", p=128),
        )
        nft_sb = work.tile([128, 8 * NB], F16, tag="nftsb", bufs=1)
        dma(
            out=nft_sb[:].rearrange("p (m d) -> p m d", m=8),
            in_=nft.ap().rearrange("(m p) d -> p m d", p=128),
        )
        x1t_sb = work.tile([128, 4 * NB], F16, tag="x1t", bufs=1)
        for mc in range(4):  # hid chunks
            ps_x1 = ps.tile([128, NB], F32, tag="big", bufs=3)
            for ki in range(8):
                mm(
                    ps_x1[:],
                    w1_sb[:, HID_DIM * ki + 128 * mc : HID_DIM * ki + 128 * mc + 128],
                    nft_sb[:, NB * ki : NB * (ki + 1)],
                    start=(ki == 0),
                    stop=(ki == 7),
                )
            act(
                x1t_sb[:, NB * mc : NB * (mc + 1)],
                ps_x1[:],
                AF.Relu,
                bias=b1_sb[:, mc : mc + 1],
            )
        w2_sb = work.tile([128, 4 * OUT_DIM], F16, tag="w2sb", bufs=1)
        dma(
            out=w2_sb[:].rearrange("p (m d) -> p m d", m=4),
            in_=w2.ap().rearrange("(m p) d -> p m d", p=128),
        )
        ps_x0 = ps.tile([128, NB], F32, tag="big", bufs=3)
        for kc in range(4):
            mm(
                ps_x0[:],
                w2_sb[:, 128 * kc : 128 * (kc + 1)],
                x1t_sb[:, NB * kc : NB * (kc + 1)],
                start=(kc == 0),
                stop=(kc == 3),
            )
        x0t16 = work.tile([128, NB], F16, tag="vt16")
        vec.tensor_scalar(x0t16[:], ps_x0[:], b2_sb[:, 0:1], None, Alu.add)

        # ---------- pe0 = tanh(pos@pe_w+pe_b) in fp32, transposed ----------
        pew_sb = work.tile([PE_IN, PE_HID], F32, tag="pewsb", bufs=1)
        dma(out=pew_sb[:], in_=pew.ap())
        post_sb = work.tile([PE_IN, NB], F32, tag="postsb", bufs=1)
        dma(out=post_sb[:], in_=postf.ap())
        ps_pe0 = ps.tile([PE_HID, NB], F32, tag="acc", bufs=2)
        mm(ps_pe0[:], pew_sb[:], post_sb[:], start=True, stop=True)
        ptf = work.tile([PE_HID, NB], F32, tag="ptf")  # pe.T fp32 state
        act(ptf[:], ps_pe0[:], AF.Tanh, bias=peb_sb[:, 0:1])
        vec.tensor_copy(raw_sb[:], ptf[:])
        pt16 = work.tile([PE_HID, NB], F16, tag="pt16")
        vec.tensor_copy(pt16[:], ptf[:])

        # ---- helpers ----
        def emit_gamma(ptf_tile, j):
            for c in range(NBC):
                ps_g = ps.tile([128, 1], F32, tag="small", bufs=3)
                mm(
                    ps_g[:],
                    ptf_tile[:, 128 * c : 128 * (c + 1)],
                    gatet_sb[:, j : j + 1],
                    start=True,
                    stop=True,
                )
                act(
                    gam[c][:, j : j + 1],
                    ps_g[:],
                    AF.Sigmoid,
                    bias=gateb_sb[0:128, j : j + 1],
                )

        def emit_ct(pt16_tile, r):
            """c^T own block from pe^T; returns ct_own2 [128,NB] and AGs if r<K."""
            ps_ct = ps.tile([PE_HID, NB], F32, tag="acc", bufs=2)
            mm(ps_ct[:], corw_sb[:], pt16_tile[:], start=True, stop=True)
            ct_own2 = work.tile([128, NB], F16, tag="ctown")
            vec.tensor_scalar(
                ct_own2[0:PE_HID, :], ps_ct[:], corb_sb[:, 0:1], None, Alu.add
            )
            dma(out=ct_own2[PE_HID:128, :], in_=ct_own2[0:PE_HID, :])
            if r < K:
                ct_agin = dram.tile([PE_HID, NB], F16, name=f"ct_agin{r}")
                dma(out=ct_agin[:], in_=ct_own2[0:PE_HID, :])
                ct_agout = dram.tile(
                    [R * PE_HID, NB], F16, addr_space="Shared", name=f"ct_agout{r}"
                )
                nc.gpsimd.collective_compute(
                    "AllGather",
                    Alu.bypass,
                    replica_groups=RG,
                    ins=[ct_agin[:].opt()],
                    outs=[ct_agout[:].opt()],
                )
                return ct_own2, ct_agout
            return ct_own2, None

        def emit_pe_shard(pt16_tile, r):
            """transpose pe^T -> natural [NB,64] fp16, AG if r<K."""
            pe_agin = dram.tile([NB, PE_HID], F16, name=f"pe_agin{r}")
            for c in range(NBC):
                ps_tr = ps.tile([128, PE_HID], F16, tag="small", bufs=3)
                nc.tensor.transpose(
                    ps_tr[:], pt16_tile[:, 128 * c : 128 * (c + 1)], id64h_sb[:]
                )
                pchunk = work.tile([128, PE_HID], F16, tag="pchunk", bufs=3)
                vec.tensor_copy(pchunk[:], ps_tr[:])
                dma(out=pe_agin[:][128 * c : 128 * (c + 1), :], in_=pchunk[:])
            pe_agout = dram.tile(
                [N, PE_HID], F16, addr_space="Shared", name=f"pe_agout{r}"
            )
            nc.gpsimd.collective_compute(
                "AllGather",
                Alu.bypass,
                replica_groups=RG,
                ins=[pe_agin[:].opt()],
                outs=[pe_agout[:].opt()],
            )
            return pe_agout

        def emit_v_shard(vt16_tile, k):
            """transpose v^T[128f,NB] -> natural chunks into v_keep[k], AG if k<K."""
            v_agin = dram.tile([NB, OUT_DIM], F16, name=f"v_agin{k}") if k < K else None
            for c in range(NBC):
                ps_tr = ps.tile([128, 128], F16, tag="small", bufs=3)
                nc.tensor.transpose(
                    ps_tr[:], vt16_tile[:, 128 * c : 128 * (c + 1)], id128h_sb[:]
                )
                vec.tensor_copy(v_keep[k][:, 128 * c : 128 * (c + 1)], ps_tr[:])
                if k < K:
                    dma(
                        out=v_agin[:][128 * c : 128 * (c + 1), :],
                        in_=v_keep[k][:, 128 * c : 128 * (c + 1)],
                    )
            if k < K:
                v_agout = dram.tile(
                    [N, OUT_DIM], F16, addr_space="Shared", name=f"v_agout{k}"
                )
                nc.gpsimd.collective_compute(
                    "AllGather",
                    Alu.bypass,
                    replica_groups=RG,
                    ins=[v_agin[:].opt()],
                    outs=[v_agout[:].opt()],
                )
                return v_agout
            return None

        # bootstrap: gamma0, v0 shard+AG, ct0, pe0 shard AG
        emit_gamma(ptf, 0)
        v_agout = emit_v_shard(x0t16, 0)
        ct_own2, ct_agout = emit_ct(pt16, 0)
        pe_agout = emit_pe_shard(pt16, 0)

        # ---------- main rounds ----------
        for r in range(K):
            # ---- Krylov application r: v_{r+1}^T = v_r^T @ Wl^T ----
            v_sb = work.tile([128, NCHUNK * 128], F16, tag="vsb")
            dma(
                out=v_sb[:].rearrange("p (m d) -> p m d", m=NCHUNK),
                in_=v_agout[:].rearrange("(m p) d -> p m d", p=128),
            )
            ps_v = ps.tile([128, NB], F32, tag="big", bufs=3)
            for m in range(NCHUNK):
                mm(
                    ps_v[:],
                    v_sb[:, 128 * m : 128 * (m + 1)],
                    wlt_sb[:, NB * m : NB * (m + 1)],
                    start=(m == 0),
                    stop=(m == NCHUNK - 1),
                )
            vt16 = work.tile([128, NB], F16, tag="vt16")
            vec.tensor_copy(vt16[:], ps_v[:])
            v_agout = emit_v_shard(vt16, r + 1)

            # ---- pe iteration r ----
            ct2 = work.tile([128, NCHUNK * 128], F16, tag="ct2")
            for half in range(2):
                dma(
                    out=ct2[64 * half : 64 * half + 64, :].rearrange(
                        "p (b d) -> p b d", b=R
                    ),
                    in_=ct_agout[:].rearrange("(b p) d -> p b d", p=PE_HID),
                )
            pe_sb = work.tile([128, NCHUNK * PE_HID], F16, tag="pesb")
            dma(
                out=pe_sb[:].rearrange("p (m d) -> p m d", m=NCHUNK),
                in_=pe_agout[:].rearrange("(m p) d -> p m d", p=128),
            )
            ps_corr = ps.tile([PE_HID, NB], F32, tag="acc", bufs=2)
            ps_tpo = ps.tile([PE_HID, NB], F32, tag="acc", bufs=2)
            for mp in range(NCHUNK // 2):
                m0, m1 = 2 * mp, 2 * mp + 1
                ps_sa = ps.tile([128, NB], F32, tag="big", bufs=3)
                ps_sb_ = ps.tile([128, NB], F32, tag="big", bufs=3)
                mm(
                    ps_sa[:],
                    ct2[0:64, 128 * m0 : 128 * (m0 + 1)],
                    ct_own2[0:64, :],
                    start=True,
                    stop=True,
                )
                mm(
                    ps_sb_[:],
                    ct2[64:128, 128 * m1 : 128 * (m1 + 1)],
                    ct_own2[64:128, :],
                    start=True,
                    stop=True,
                )
                s16a = work.tile([128, NB], F16, tag="s16", bufs=3)
                s16b = work.tile([128, NB], F16, tag="s16", bufs=3)
                act(s16a[:], ps_sa[:], AF.Sigmoid)
                act(s16b[:], ps_sb_[:], AF.Sigmoid)
                for m, s16 in ((m0, s16a), (m1, s16b)):
                    mm(
                        ps_corr[:],
                        pe_sb[:, PE_HID * m : PE_HID * (m + 1)],
                        s16[:],
                        start=(m == 0),
                        stop=(m == NCHUNK - 1),
                    )
                    mm(
                        ps_tpo[:],
                        pe_sb[:, PE_HID * m : PE_HID * (m + 1)],
                        wgt_sb[:, NB * m : NB * (m + 1)],
                        start=(m == 0),
                        stop=(m == NCHUNK - 1),
                    )
            # pe_new = tanh(0.1*raw + 1.35*tpo - 0.45*corr)
            tnew = work.tile([PE_HID, NB], F32, tag="tnew")
            vec.tensor_scalar(tnew[:], ps_tpo[:], 1.0 + PE_BETA, None, Alu.mult)
            vec.scalar_tensor_tensor(
                tnew[:], ps_corr[:], -PE_BETA, tnew[:], Alu.mult, Alu.add
            )
            vec.scalar_tensor_tensor(
                tnew[:], raw_sb[:], PE_ALPHA / (1.0 - PE_ALPHA), tnew[:],
                Alu.mult, Alu.add,
            )
            ptf = work.tile([PE_HID, NB], F32, tag="ptf")
            act(ptf[:], tnew[:], AF.Tanh, scale=1.0 - PE_ALPHA)
            pt16 = work.tile([PE_HID, NB], F16, tag="pt16")
            vec.tensor_copy(pt16[:], ptf[:])
            emit_gamma(ptf, r + 1)
            ct_own2, ct_agout = emit_ct(pt16, r + 1)
            if r < K - 1:
                pe_agout = emit_pe_shard(pt16, r + 1)

        # ---------- final combine: out[rows] = sum_k u_k * v_k ----------
        for c in range(NBC):
            u_c = work.tile([128, K + 1], F32, tag="uc", bufs=2)
            vec.memset(u_c[:], 0.0)
            for j in range(K + 1):
                vec.scalar_tensor_tensor(
                    u_c[:],
                    mrows_sb[:, (K + 1) * j : (K + 1) * (j + 1)],
                    gam[c][:, j : j + 1],
                    u_c[:],
                    Alu.mult,
                    Alu.add,
                )
            out_c = work.tile([128, OUT_DIM], F32, tag="outc", bufs=2)
            vec.memset(out_c[:], 0.0)
            for k in range(K + 1):
                vec.scalar_tensor_tensor(
                    out_c[:],
                    v_keep[k][:, 128 * c : 128 * (c + 1)],
                    u_c[:, k : k + 1],
                    out_c[:],
                    Alu.mult,
                    Alu.add,
                )
            dma(out=out_x.ap()[128 * c : 128 * (c + 1), :], in_=out_c[:])

        # ---------- final pe output (fp32) ----------
        for c in range(NBC):
            ps_trf = ps.tile([128, PE_HID], F32, tag="small", bufs=3)
            nc.tensor.transpose(
                ps_trf[:], ptf[:, 128 * c : 128 * (c + 1)], id64f_sb[:]
            )
            pe_out_c = work.tile([128, PE_HID], F32, tag="peoutc", bufs=2)
            vec.tensor_copy(pe_out_c[:], ps_trf[:])
            dma(out=out_pe.ap()[128 * c : 128 * (c + 1), :], in_=pe_out_c[:])

    nc.compile()
    return nc


def _host_prep(node_feat, edge_index, pos_enc, lin1_w, lin1_b, lin2_w, lin2_b,
               pe_w, pe_b, cor_w, cor_b, gate_w, gate_b, temp):
    node_feat = np.asarray(node_feat, dtype=np.float32)
    edge_index = np.asarray(edge_index)
    pos_enc = np.asarray(pos_enc, dtype=np.float32)
    src = edge_index[0].astype(np.int64)
    dst = edge_index[1].astype(np.int64)

    deg = np.bincount(src, minlength=N).astype(np.float32)
    dinv = np.where(deg > 0, deg**-0.5, 0.0).astype(np.float32)
    wl = dinv[src] * dinv[dst]
    deg2 = (np.bincount(dst, minlength=N) + 1.0).astype(np.float32)
    d2 = deg2**-0.5
    wg = (d2[src] * d2[dst]).astype(np.float32)
    dd = (d2 * d2).astype(np.float32)

    Wl = np.zeros((N, N), np.float32)
    np.add.at(Wl, (dst, src), wl)
    Wg = np.zeros((N, N), np.float32)
    np.add.at(Wg, (dst, src), wg)
    Wg[np.arange(N), np.arange(N)] += dd

    WlT = np.ascontiguousarray(Wl.T).astype(np.float16)
    WgT = np.ascontiguousarray(Wg.T).astype(np.float16)

    # Bernstein-in-monomial coefficients and gamma fold matrix
    C = np.zeros((K + 1, K + 1))
    for j in range(K + 1):
        p = np.array([1.0])
        for _ in range(j):
            p = np.convolve(p, [1.0, -1.0])
        for _ in range(K - j):
            p = np.convolve(p, [1.0, 1.0])
        C[j, : len(p)] = p
    coeff = np.array([math.comb(K, j) / 2**K for j in range(K + 1)])
    TEMP = np.maximum(np.asarray(temp, np.float32), 0.0)
    Mt = (TEMP[:, None] * coeff[:, None] * C).astype(np.float32)  # [j,k]
    mrows = np.tile(Mt.reshape(1, -1), (128, 1)).astype(np.float32)

    lin1_b = np.asarray(lin1_b, np.float32)
    b1c = np.ascontiguousarray(lin1_b.reshape(HID_DIM // 128, 128).T)
    common = {
        "w1": np.asarray(lin1_w).astype(np.float16),
        "w2": np.asarray(lin2_w).astype(np.float16),
        "pew": np.asarray(pe_w, np.float32),
        "corw": np.asarray(cor_w).astype(np.float16),
        "gatet": np.ascontiguousarray(np.asarray(gate_w, np.float32).T),
        "b1c": b1c,
        "b2c": np.tile(np.asarray(lin2_b, np.float32).reshape(128, 1), (1, 1)),
        "pebc": np.asarray(pe_b, np.float32).reshape(PE_HID, 1),
        "corbc": np.asarray(cor_b, np.float32).reshape(PE_HID, 1),
        "gatebbc": np.tile(np.asarray(gate_b, np.float32).reshape(1, K + 1), (128, 1)),
        "mrows": mrows,
        "id128h": np.eye(128, dtype=np.float16),
        "id64h": np.eye(64, dtype=np.float16),
        "id64f": np.eye(64, dtype=np.float32),
    }
    in_maps = []
    for b in range(R):
        rows = slice(NB * b, NB * (b + 1))
        m = dict(common)
        m["wlt"] = np.ascontiguousarray(WlT[:, rows])
        m["wgt"] = np.ascontiguousarray(WgT[:, rows])
        m["nft"] = np.ascontiguousarray(node_feat[rows].T.astype(np.float16))
        m["postf"] = np.ascontiguousarray(pos_enc[rows].T)
        in_maps.append(m)
    return in_maps


def kernel(**inputs):
    global _last_exec_ns
    _install_ntff_hook()
    from concourse import bass_utils

    if "nc" not in _cache:
        _cache["nc"] = _build()
    nc = _cache["nc"]
    in_maps = _host_prep(**inputs)
    trace = os.environ.get("GNN_TRACE", "0") == "1"
    res = bass_utils.run_bass_kernel_spmd(
        nc, in_maps, core_ids=list(range(R)), trace=trace
    )
    _last_exec_ns = res.exec_time_ns
    out = np.concatenate([res.results[b]["out_x"] for b in range(R)], axis=0)
    pe = np.concatenate([res.results[b]["out_pe"] for b in range(R)], axis=0)
    return out, pe
